# revision 1
# baseline (speedup 1.0000x reference)
import numpy as np
import jax
import jax.numpy as jnp
from functools import partial

# Problem dims (nn_LocalMultiHeadChannelAttention): B=16, C=512, R=32, PS=3,
# HN=8, D=128, H=W=96. Sharding: data-parallel over batch B across 8 cores.
B, C, R, PS, HN, D = 16, 512, 32, 3, 8, 128
NORM_C = 0.5
NCORES = 8


def _avg_pool(x, k):
    b, c, h, w = x.shape
    return x.reshape(b, c, h // k, k, w // k, k).mean(axis=(3, 5))


def _max_pool(x, k):
    b, c, h, w = x.shape
    return x.reshape(b, c, h // k, k, w // k, k).max(axis=(3, 5))


def _to_heads(p, b):
    t = p.reshape(b, R * R, C).transpose(0, 2, 1)
    return t.reshape(b, C, HN, D).transpose(0, 2, 1, 3)  # [b, HN, C, D]


def _shard_body(x, Wqk, bqk, Wp, bp, Wv, bv, wscale):
    # x: [B/NCORES, C, H, H] on one core; params replicated.
    b = x.shape[0]
    q_pool = _avg_pool(x, PS)                     # [b, C, R, R]
    k_pool = _max_pool(x, PS)

    q = jnp.einsum('bhcd,hed->bhce', _to_heads(q_pool, b), Wqk) + bqk[None, :, None, :]
    k = jnp.einsum('bhcd,hed->bhce', _to_heads(k_pool, b), Wqk) + bqk[None, :, None, :]

    # 1x1 conv commutes with avg-pool: avg_pool(Wv@x + bv) == Wv@avg_pool(x) + bv
    v_conv = jnp.einsum('bchw,oc->bohw', q_pool[:, :, :, :, None, None]
                        .reshape(b, C, R, R), Wv) + bv[None, :, None, None]
    v = _to_heads(v_conv, b)

    scores = jnp.einsum('bhcd,bhed->bhce', q, k)
    p = jax.nn.sigmoid(scores.mean(axis=-1) @ Wp.T + bp)
    norm_scores = scores / jnp.power(jnp.float32(D), NORM_C + p[..., None])
    w = jax.nn.softmax(norm_scores, axis=-1)
    attn = jnp.einsum('bhce,bhed->bhcd', w, v)

    attn = attn.transpose(0, 2, 1, 3).reshape(b, C, R * R)
    attn = attn.transpose(0, 2, 1).reshape(b, R, R, C)
    resid = q_pool.reshape(b, R * R, C).reshape(b, R, R, C)
    return resid + attn * wscale


def kernel(x, Wqk, bqk, Wp, bp, Wv, bv, weight):
    x = np.asarray(x, dtype=np.float32)
    wscale = np.float32(1 + int(np.asarray(weight)))
    params = tuple(np.asarray(t, dtype=np.float32) for t in (Wqk, bqk, Wp, bp, Wv, bv))

    xs = x.reshape(NCORES, B // NCORES, C, PS * R, PS * R)
    fn = jax.pmap(partial(_shard_body, wscale=wscale),
                  in_axes=(0,) + (None,) * 6, devices=jax.devices()[:NCORES])
    out = fn(xs, *params)                          # [NCORES, B/NCORES, R, R, C]
    return np.asarray(out).reshape(B, R, R, C).astype(np.float32)


# revision 2
# speedup vs baseline: 20.4983x; 20.4983x over previous
import numpy as np
import jax
import jax.numpy as jnp
from functools import lru_cache, partial

# nn_LocalMultiHeadChannelAttention: B=16, C=512, R=32, PS=3, HN=8, D=128,
# input spatial H=W=96. Sharded data-parallel over batch B across 8 cores
# (2 batches/core); all params replicated. No collectives needed.
B, C, R, PS, HN, D = 16, 512, 32, 3, 8, 128
NORM_C = 0.5
NCORES = 8


def _to_heads(p, b):
    # [b,C,R,R] -> [b,HN,C,D] via the reference's reshape/permute chain
    t = p.reshape(b, R * R, C).transpose(0, 2, 1)
    return t.reshape(b, C, HN, D).transpose(0, 2, 1, 3)


def _shard_body(x, Wqk, bqk, Wp, bp, Wv, bv, wscale):
    b = x.shape[0]
    # 3x3 pools; H=96 -> R=32
    xr = x.reshape(b, C, R, PS, R, PS)
    q_pool = xr.mean(axis=(3, 5))            # [b, C, R, R]
    k_pool = xr.max(axis=(3, 5))

    q = jnp.einsum('bhcd,hed->bhce', _to_heads(q_pool, b), Wqk) + bqk[None, :, None, :]
    k = jnp.einsum('bhcd,hed->bhce', _to_heads(k_pool, b), Wqk) + bqk[None, :, None, :]

    # 1x1 conv commutes with avg-pool: avg_pool3(Wv@x + bv) == Wv@q_pool + bv
    v_conv = jnp.einsum('bchw,oc->bohw', q_pool, Wv) + bv[None, :, None, None]
    v = _to_heads(v_conv, b)

    scores = jnp.einsum('bhcd,bhed->bhce', q, k)          # [b,HN,C,C]
    p = jax.nn.sigmoid(scores.mean(axis=-1) @ Wp.T + bp)  # [b,HN,C]
    norm_scores = scores / jnp.power(jnp.float32(D), NORM_C + p[..., None])
    w = jax.nn.softmax(norm_scores, axis=-1)
    attn = jnp.einsum('bhce,bhed->bhcd', w, v)

    attn = attn.transpose(0, 2, 1, 3).reshape(b, C, R * R)
    attn = attn.transpose(0, 2, 1).reshape(b, R, R, C)
    resid = q_pool.reshape(b, R * R, C).reshape(b, R, R, C)
    return resid + attn * wscale


@lru_cache(maxsize=4)
def _build(wscale):
    return jax.pmap(partial(_shard_body, wscale=np.float32(wscale)),
                    in_axes=(0,) + (None,) * 6,
                    devices=jax.devices()[:NCORES])


def kernel(x, Wqk, bqk, Wp, bp, Wv, bv, weight):
    x = np.asarray(x, dtype=np.float32)
    wscale = float(1 + int(np.asarray(weight)))
    params = tuple(np.asarray(t, dtype=np.float32) for t in (Wqk, bqk, Wp, bp, Wv, bv))

    xs = x.reshape(NCORES, B // NCORES, C, PS * R, PS * R)
    out = _build(wscale)(xs, *params)        # [NCORES, B/NCORES, R, R, C]
    return np.asarray(out).reshape(B, R, R, C).astype(np.float32)
